# revision 2
# baseline (speedup 1.0000x reference)
"""Trainium2 Bass kernel for the BiaffineLayer problem.

Math (per batch b):
  out[l, m, c] = x1[l] @ W1[c] + x2[m] @ W2[c]
              + sum_h x1[l,h] * x2[m,h] * W3[c,h]
              + sum_h |x1[l,h] - x2[m,h]| * W4[c,h] + bias[c]
  shapes: x1, x2 [2, 512, 128]; W [25, 512]; bias [25]; out [2, 512, 512, 25]

Sharding: 8 cores = 2 batches x 4 m-blocks of 128 columns. Core (b, mb) gets
full x1[b] and its x2[b, m0:m0+128] block; it produces out[b, :, m0:m0+MB, :].

Decomposition, with |d| = 2*relu(d) - d and d = x1 - x2[m]:
  out = x1t' @ V3  +  D_m' @ (2 W4T)  +  T2B
where (host-precomputed except D):
  V3[h,(m,c)] = x2[m,h]*W3[c,h] + (W1-W4)[c,h]   (t3 + t1 - t4's -x1*W4 part)
  T2B[m,c]    = x2[m] @ (W2+W4)T + b             (added on the HOST during
                                                  unshard - pure per-(m,c))
  D_m[h,l]    = relu(x1[l,h] - x2[m,h])          (device, pairwise)

Measured HW rates (microbenched): DVE tensor_scalar = 129ns + 0.26ns/elem
(~263ns per [128,512] D tile); ACT relu-with-bias ~612ns/tile, ACT PSUM
drain [128,800] ~929ns; PE t4 LDW+MM pair 25ns, t3 400-col MM 168ns;
gpsimd compute useless (7.5us/op) but fine as a DMA-issue queue (~700ns).
D-gen (DVE+ACT saturated) is the critical path at ~28us; PE totals ~18us.

v2 schedule changes vs v1:
  - input DMAs issue on parallel queues: sync: x1t -> (w4t2|v3a) -> v3b;
    gpsimd: negx2. D-gen starts ~2us earlier (negx2 no longer serialized
    behind x1t on one queue).
  - w4t2 and v3 merged into one dram tensor (w4t2 = first 25 cols) to
    save a DMA issue.
  - out-DMAs issue from the gpsimd queue (idle otherwise); the final
    block's two drain-half DMAs split across gpsimd+sync.
  - DVE/ACT balance: 105 D tiles on DVE, 23 on ACT (block 0 gives ACT 5
    tiles since it has no drain work yet), drains on ACT one block late;
    last drain split DVE+ACT.
  - dpool bufs reduced 128->32 (fewer allocated semaphores -> shorter
    framework teardown chain).
"""

import sys

sys.path.insert(0, "/opt/trn_rl_repo")

from contextlib import ExitStack

import ml_dtypes
import numpy as np

import concourse.bass as bass
import concourse.tile as tile
from concourse import bacc, bass_utils, mybir

F32 = mybir.dt.float32
BF16 = mybir.dt.bfloat16
BF16_NP = ml_dtypes.bfloat16

B, L, H, C = 2, 512, 128, 25
MB = 128            # m-block per core
N_CORES = 8
MSUB = 16           # m's per psum block
N_MS = MB // MSUB   # 8 blocks over the m-block
LCHUNK = 128
N_LC = L // LCHUNK  # 4 l-chunks
CHUNK_F = MSUB * C  # 400 psum free columns per l-chunk slice
PS_STRIDE = 512     # psum bank stride (f32 elems) per l-chunk slice

# Per-block D-tile engine split: V=DVE, A=ACT. Totals: DVE 105, ACT 23.
# Block 0 has no preceding drain, so ACT takes 5 tiles there (placed late
# in j order so the PE's j-major consumption hits DVE tiles first).
D_PATS = {
    11: "VVVVVVVAVAVAVAVA",   # V=11 A=5 (block 0; A late-ish, interleaved)
    13: "VVAVVVVVAVVVVVAV",   # V=13 A=3
    14: "VVAVVVVVVVAVVVVV",   # V=14 A=2
}
D_NV = [11, 13, 13, 13, 14, 13, 14, 14]   # sum V = 105


def build_kernel(nc: bass.Bass, repeat: int = 1):
    x1t = nc.dram_tensor("x1t", (H, L), BF16, kind="ExternalInput").ap()
    negx2 = nc.dram_tensor("negx2", (H, MB), F32, kind="ExternalInput").ap()
    # w4t2 (25 cols) | v3 (3200 cols) merged
    wv3 = nc.dram_tensor("wv3", (H, C + MB * C), BF16, kind="ExternalInput").ap()
    out = nc.dram_tensor("out", (L, MB * C), BF16, kind="ExternalOutput").ap()

    with tile.TileContext(nc) as tc, ExitStack() as ctx:
      const = ctx.enter_context(tc.tile_pool(name="const", bufs=1))
      dpool = ctx.enter_context(tc.tile_pool(name="dpool", bufs=32))
      opool = ctx.enter_context(tc.tile_pool(name="opool", bufs=8))
      psum = ctx.enter_context(tc.tile_pool(name="psum", bufs=4, space="PSUM"))
      for _rep in range(repeat):
        # ---- input loads: parallel issue on two queues ----
        x1t_tile = const.tile([H, L], BF16)
        nc.sync.dma_start(x1t_tile[:], x1t[:])
        negx2_tile = const.tile([H, MB], F32)
        nc.gpsimd.dma_start(negx2_tile[:], negx2[:])
        x1t_bf = x1t_tile[:]
        negx2_f = negx2_tile[:]
        wv3_tile = const.tile([H, C + MB * C], BF16)
        w4t2_bf = wv3_tile[:, 0:C]
        v3_bf = wv3_tile[:, C:]
        # w4t2 + first 2 blocks of v3 now; rest staged at ms==1
        nc.sync.dma_start(wv3_tile[:, 0 : C + 2 * CHUNK_F],
                          wv3[:, 0 : C + 2 * CHUNK_F])
        ones_bf = const.tile([1, LCHUNK], BF16)
        nc.vector.memset(ones_bf[:], 1.0)
        # preload the ACT activation table off the critical path
        act_warm = const.tile([1, LCHUNK], BF16)
        nc.scalar.activation(act_warm[:], ones_bf[:],
                             mybir.ActivationFunctionType.Relu)

        # ---- main loop over m-blocks ----
        # Drains emitted one block late so they sit behind the next block's
        # D work in the ACT queue (prioritizing D production).
        pend = None

        def emit_drain(p):
            ms_, psa3_, psb3_, last = p
            o_sb = opool.tile([LCHUNK, N_LC * CHUNK_F], BF16)
            o3 = o_sb[:].rearrange("p (lc c) -> p lc c", c=CHUNK_F)
            out3 = (out[:, ms_ * CHUNK_F : (ms_ + 1) * CHUNK_F]
                    .rearrange("(lc p) c -> p lc c", p=LCHUNK))
            if not last:
                # both halves on ACT keep DVE free for D tiles
                nc.scalar.copy(o3[:, 0:2], psa3_)
                nc.scalar.copy(o3[:, 2:4], psb3_)
                nc.gpsimd.dma_start(out3, o3)
            else:  # split across engines + 2 DMAs on 2 queues: short tail
                nc.vector.tensor_copy(o3[:, 0:2], psa3_)
                nc.sync.dma_start(out3[:, 0:2], o3[:, 0:2])
                nc.scalar.copy(o3[:, 2:4], psb3_)
                nc.gpsimd.dma_start(out3[:, 2:4], o3[:, 2:4])

        for ms in range(N_MS):
            if ms == 1:
                # late-need v3 bulk, emitted after block 0's D ops so the
                # early compute's DMA-sem gates stay on x1t+negx2 only
                nc.sync.dma_start(
                    wv3_tile[:, C + 2 * CHUNK_F :], wv3[:, C + 2 * CHUNK_F :])
            # D tiles for this block
            dts = []
            pat = D_PATS[D_NV[ms]]
            for j in range(MSUB):
                m = ms * MSUB + j
                dt_ = dpool.tile([H, L], BF16, tag="d")
                if pat[j] == "V":
                    nc.vector.tensor_scalar(
                        dt_[:], x1t_bf, negx2_f[:, m : m + 1], 0.0,
                        op0=mybir.AluOpType.add, op1=mybir.AluOpType.max)
                else:
                    nc.scalar.activation(
                        dt_[:], x1t_bf, mybir.ActivationFunctionType.Relu,
                        bias=negx2_f[:, m : m + 1], scale=1.0)
                dts.append(dt_)

            if pend is not None:
                emit_drain(pend)

            ps_a = psum.tile([LCHUNK, 2 * PS_STRIDE], F32, tag="ps")
            ps_b = psum.tile([LCHUNK, 2 * PS_STRIDE], F32, tag="ps")
            pss = [ps_a, ps_a, ps_b, ps_b]
            # t3 (+t1 fold) opens each group full-width (PSUM zeroing is
            # bank-granular), then the t4s accumulate j-major.
            for lc in range(N_LC):
                nc.tensor.matmul(
                    pss[lc][:, (lc % 2) * PS_STRIDE :
                            (lc % 2) * PS_STRIDE + CHUNK_F],
                    x1t_bf[:, lc * LCHUNK : (lc + 1) * LCHUNK],
                    v3_bf[:, ms * CHUNK_F : (ms + 1) * CHUNK_F],
                    start=True, stop=False, skip_group_check=True)
            for j in range(MSUB):
                for lc in range(N_LC):
                    base = (lc % 2) * PS_STRIDE
                    nc.tensor.matmul(
                        pss[lc][:, base + j * C : base + (j + 1) * C],
                        dts[j][:, lc * LCHUNK : (lc + 1) * LCHUNK],
                        w4t2_bf[:],
                        start=False, stop=(j == MSUB - 1),
                        skip_group_check=True)

            psa3 = ps_a[:].rearrange("p (lc x) -> p lc x",
                                     x=PS_STRIDE)[:, :, 0:CHUNK_F]
            psb3 = ps_b[:].rearrange("p (lc x) -> p lc x",
                                     x=PS_STRIDE)[:, :, 0:CHUNK_F]
            pend = (ms, psa3, psb3, ms == N_MS - 1)
        emit_drain(pend)
    return nc


_COMPILED = {}


def _get_compiled():
    if "nc" not in _COMPILED:
        nc = bacc.Bacc("TRN2", target_bir_lowering=False, debug=False,
                       num_devices=N_CORES)
        build_kernel(nc)
        nc.compile()
        _COMPILED["nc"] = nc
    return _COMPILED["nc"]


def make_in_maps(x1, x2, W, b):
    W1, W2, W3, W4 = (W[:, 0:H], W[:, H : 2 * H], W[:, 2 * H : 3 * H],
                      W[:, 3 * H : 4 * H])
    w13 = (W1 - W4).T.astype(np.float32)          # [H, C]
    w3t = W3.T.astype(np.float32)                 # [H, C]
    w4t2 = (2.0 * W4).T.astype(np.float32)        # [H, C]
    in_maps = []
    for cid in range(N_CORES):
        bb, mblk = cid // 4, cid % 4
        m0 = mblk * MB
        x2blk = x2[bb, m0 : m0 + MB]              # [MB, H]
        x2t = x2blk.T                             # [H, MB]
        # V3[h, m*C+c] = x2t[h,m]*W3T[h,c] + (W1-W4)T[h,c]
        v3 = x2t[:, :, None] * w3t[:, None, :] + w13[:, None, :]
        wv3 = np.concatenate([w4t2, v3.reshape(H, MB * C)], axis=1)
        in_maps.append({
            "x1t": np.ascontiguousarray(x1[bb].T.astype(BF16_NP)),
            "negx2": np.ascontiguousarray(-x2t.astype(np.float32)),
            "wv3": np.ascontiguousarray(wv3.astype(BF16_NP)),
        })
    return in_maps


def t2_bias(x2, W, b):
    """Host-side t2 term: x2 @ (W2+W4).T + bias, [B, L, C] f32."""
    W2 = W[:, H : 2 * H]
    W4 = W[:, 3 * H : 4 * H]
    return (x2 @ (W2 + W4).T + b).astype(np.float32)


def run_on_device(x1, x2, W, b, trace=False, trace_kwargs=None):
    nc = _get_compiled()
    in_maps = make_in_maps(x1, x2, W, b)
    res = bass_utils.run_bass_kernel_spmd(
        nc, in_maps, core_ids=list(range(N_CORES)), trace=trace,
        **(trace_kwargs or {}))
    t2 = t2_bias(x2, W, b)                        # [B, L, C]
    full = np.empty((B, L, L, C), dtype=np.float32)
    for cid in range(N_CORES):
        bb, mblk = cid // 4, cid % 4
        m0 = mblk * MB
        full[bb, :, m0 : m0 + MB, :] = (
            np.asarray(res.results[cid]["out"])
            .astype(np.float32).reshape(L, MB, C)
            + t2[bb, m0 : m0 + MB, :][None, :, :])
    return full, res


def kernel(x1, x2, W, b):
    x1 = np.asarray(x1, dtype=np.float32)
    x2 = np.asarray(x2, dtype=np.float32)
    W = np.asarray(W, dtype=np.float32)
    b = np.asarray(b, dtype=np.float32)
    full, _ = run_on_device(x1, x2, W, b, trace=False)
    return full


# revision 5
# speedup vs baseline: 1.0415x; 1.0415x over previous
"""Trainium2 Bass kernel for the BiaffineLayer problem.

Math (per batch b):
  out[l, m, c] = x1[l] @ W1[c] + x2[m] @ W2[c]
              + sum_h x1[l,h] * x2[m,h] * W3[c,h]
              + sum_h |x1[l,h] - x2[m,h]| * W4[c,h] + bias[c]
  shapes: x1, x2 [2, 512, 128]; W [25, 512]; bias [25]; out [2, 512, 512, 25]

Sharding: 8 cores = 2 batches x 4 m-blocks of 128 columns. Core (b, mb) gets
full x1[b] and its x2[b, m0:m0+128] block; it produces out[b, :, m0:m0+MB, :].

Decomposition, with |d| = 2*relu(d) - d and d = x1 - x2[m]:
  out = x1t' @ V3  +  D_m' @ (2 W4T)  +  T2B
where (host-precomputed except D):
  V3[h,(m,c)] = x2[m,h]*W3[c,h] + (W1-W4)[c,h]   (t3 + t1 - t4's -x1*W4 part)
  T2B[m,c]    = x2[m] @ (W2+W4)T + b             (added on the HOST during
                                                  unshard - pure per-(m,c))
  D_m[h,l]    = relu(x1[l,h] - x2[m,h])          (device, pairwise)

Measured HW rates (microbenched): DVE tensor_scalar = 129ns + 0.26ns/elem
(~263ns per [128,512] D tile, 2x bf16 mode already engaged); ACT relu ~612ns
effective, ACT [128,1600] PSUM drain ~1480ns; PE t4 LDW+MM pair 25ns, t3
400-col MM 168ns (PE total ~18us, not critical); input DMA is descriptor-
rate-bound (~17ns per 1KB partition row -> 2.2us for x1t on one queue);
gpsimd compute useless (7.5us/op); large out-DMAs on the gpsimd queue hit a
~19us descriptor pathology - keep them on sync.

Schedule (v3):
  - input DMAs split across queues to parallelize descriptor processing:
    sync: x1t[:,0:256] -> wv3a(w4t2+v3 blocks 0-1) -> out-DMAs
    vector: x1t[:,256:512] -> wv3b(v3 blocks 2-4)
    gpsimd: negx2[:,0:64] -> negx2[:,64:128] -> wv3c(v3 blocks 5-7)
    D-gen starts ~9.1us instead of ~10.6.
  - single 4-bank PSUM tile per block (bufs=2), drain = ONE ACT op
    [128,4,400] per block, emitted one block late; last block's drain
    split DVE+ACT with its two out-DMAs on sync+tensor queues.
  - D-tile engine split DVE 102 / ACT 26 (block 0-1 give ACT 4 tiles as
    they carry no/less drain work).
"""

import sys

sys.path.insert(0, "/opt/trn_rl_repo")

from contextlib import ExitStack

import ml_dtypes
import numpy as np

import concourse.bass as bass
import concourse.tile as tile
from concourse import bacc, bass_utils, mybir

F32 = mybir.dt.float32
BF16 = mybir.dt.bfloat16
BF16_NP = ml_dtypes.bfloat16

B, L, H, C = 2, 512, 128, 25
MB = 128            # m-block per core
N_CORES = 8
MSUB = 16           # m's per psum block
N_MS = MB // MSUB   # 8 blocks over the m-block
LCHUNK = 128
N_LC = L // LCHUNK  # 4 l-chunks
CHUNK_F = MSUB * C  # 400 psum free columns per l-chunk slice
PS_STRIDE = 512     # psum bank stride (f32 elems) per l-chunk slice

# Per-block D-tile engine split: V=DVE, A=ACT. sum V = 102.
D_PATS = {
    12: "VVVAVVVAVVVAVVVA",   # V=12 A=4
    13: "VVAVVVVVAVVVVVAV",   # V=13 A=3
}
D_NV = [12, 12, 13, 13, 13, 13, 13, 13]   # sum = 102


def build_kernel(nc: bass.Bass, repeat: int = 1):
    x1t = nc.dram_tensor("x1t", (H, L), BF16, kind="ExternalInput").ap()
    negx2 = nc.dram_tensor("negx2", (H, MB), F32, kind="ExternalInput").ap()
    # w4t2 (25 cols) | v3 (3200 cols) merged
    wv3 = nc.dram_tensor("wv3", (H, C + MB * C), BF16, kind="ExternalInput").ap()
    out = nc.dram_tensor("out", (L, MB * C), BF16, kind="ExternalOutput").ap()

    WVA = C + 2 * CHUNK_F        # w4t2 + v3 blocks 0-1
    WVB = C + 5 * CHUNK_F        # ... through block 4

    with tile.TileContext(nc) as tc, ExitStack() as ctx:
      const = ctx.enter_context(tc.tile_pool(name="const", bufs=1))
      dpool = ctx.enter_context(tc.tile_pool(name="dpool", bufs=128))
      opool = ctx.enter_context(tc.tile_pool(name="opool", bufs=8))
      psum = ctx.enter_context(tc.tile_pool(name="psum", bufs=2, space="PSUM"))
      for _rep in range(repeat):
        # ---- input loads ----
        # DMA queues are sync/scalar/gpsimd only. Input latency is
        # descriptor-rate-bound (~17ns per 1KB partition row), so the
        # critical x1t+negx2 bytes are split across all three rings; the
        # scalar ring processes descriptors while the ACT engine loads its
        # activation table (act_warm), so that queue costs ACT nothing.
        x1t_tile = const.tile([H, L], BF16)
        negx2_tile = const.tile([H, MB], F32)
        x1t_bf = x1t_tile[:]
        negx2_f = negx2_tile[:]
        wv3_tile = const.tile([H, C + MB * C], BF16)
        w4t2_bf = wv3_tile[:, 0:C]
        v3_bf = wv3_tile[:, C:]
        ones_bf = const.tile([1, LCHUNK], BF16)

        nc.scalar.dma_start(x1t_tile[:, 384:512], x1t[:, 384:512])
        nc.sync.dma_start(x1t_tile[:, 0:256], x1t[:, 0:256])
        nc.gpsimd.dma_start(negx2_tile[:, 0:64], negx2[:, 0:64])
        nc.gpsimd.dma_start(x1t_tile[:, 256:384], x1t[:, 256:384])
        nc.gpsimd.dma_start(negx2_tile[:, 64:128], negx2[:, 64:128])
        nc.sync.dma_start(wv3_tile[:, 0:WVA], wv3[:, 0:WVA])
        nc.gpsimd.dma_start(wv3_tile[:, WVB:], wv3[:, WVB:])
        nc.vector.memset(ones_bf[:], 1.0)
        # preload the ACT activation table; then the scalar ring fetches
        # v3 blocks 2-4 (needed ~17us, lands ~13us)
        act_warm = const.tile([1, LCHUNK], BF16)
        nc.scalar.activation(act_warm[:], ones_bf[:],
                             mybir.ActivationFunctionType.Relu)
        nc.scalar.dma_start(wv3_tile[:, WVA:WVB], wv3[:, WVA:WVB])

        # ---- main loop over m-blocks ----
        # Drains (one ACT op each) emitted one block late so they queue
        # behind the next block's D work, prioritizing D production.
        pend = None

        def emit_drain(p):
            ms_, ps3_, last = p
            o_sb = opool.tile([LCHUNK, N_LC * CHUNK_F], BF16)
            o3 = o_sb[:].rearrange("p (lc c) -> p lc c", c=CHUNK_F)
            out3 = (out[:, ms_ * CHUNK_F : (ms_ + 1) * CHUNK_F]
                    .rearrange("(lc p) c -> p lc c", p=LCHUNK))
            if not last:
                nc.scalar.copy(o3[:], ps3_)
                nc.sync.dma_start(out3, o3)
            else:  # split engines + 2 DMAs on 2 queues for a short tail
                nc.vector.tensor_copy(o3[:, 0:2], ps3_[:, 0:2])
                nc.sync.dma_start(out3[:, 0:2], o3[:, 0:2])
                nc.scalar.copy(o3[:, 2:4], ps3_[:, 2:4])
                nc.scalar.dma_start(out3[:, 2:4], o3[:, 2:4])

        for ms in range(N_MS):
            # D tiles for this block
            dts = []
            pat = D_PATS[D_NV[ms]]
            for j in range(MSUB):
                m = ms * MSUB + j
                dt_ = dpool.tile([H, L], BF16, tag="d")
                if pat[j] == "V":
                    nc.vector.tensor_scalar(
                        dt_[:], x1t_bf, negx2_f[:, m : m + 1], 0.0,
                        op0=mybir.AluOpType.add, op1=mybir.AluOpType.max)
                else:
                    nc.scalar.activation(
                        dt_[:], x1t_bf, mybir.ActivationFunctionType.Relu,
                        bias=negx2_f[:, m : m + 1], scale=1.0)
                dts.append(dt_)

            if pend is not None:
                emit_drain(pend)

            ps = psum.tile([LCHUNK, N_LC * PS_STRIDE], F32, tag="ps")
            # t3 (+t1 fold) opens each group full-width (PSUM zeroing is
            # bank-granular), then the t4s accumulate j-major.
            for lc in range(N_LC):
                nc.tensor.matmul(
                    ps[:, lc * PS_STRIDE : lc * PS_STRIDE + CHUNK_F],
                    x1t_bf[:, lc * LCHUNK : (lc + 1) * LCHUNK],
                    v3_bf[:, ms * CHUNK_F : (ms + 1) * CHUNK_F],
                    start=True, stop=False, skip_group_check=True)
            for j in range(MSUB):
                for lc in range(N_LC):
                    base = lc * PS_STRIDE
                    nc.tensor.matmul(
                        ps[:, base + j * C : base + (j + 1) * C],
                        dts[j][:, lc * LCHUNK : (lc + 1) * LCHUNK],
                        w4t2_bf[:],
                        start=False, stop=(j == MSUB - 1),
                        skip_group_check=True)

            ps3 = ps[:].rearrange("p (lc x) -> p lc x",
                                  x=PS_STRIDE)[:, :, 0:CHUNK_F]
            pend = (ms, ps3, ms == N_MS - 1)
        emit_drain(pend)
    return nc


_COMPILED = {}


def _get_compiled():
    if "nc" not in _COMPILED:
        nc = bacc.Bacc("TRN2", target_bir_lowering=False, debug=False,
                       num_devices=N_CORES)
        build_kernel(nc)
        nc.compile()
        _COMPILED["nc"] = nc
    return _COMPILED["nc"]


def make_in_maps(x1, x2, W, b):
    W1, W2, W3, W4 = (W[:, 0:H], W[:, H : 2 * H], W[:, 2 * H : 3 * H],
                      W[:, 3 * H : 4 * H])
    w13 = (W1 - W4).T.astype(np.float32)          # [H, C]
    w3t = W3.T.astype(np.float32)                 # [H, C]
    w4t2 = (2.0 * W4).T.astype(np.float32)        # [H, C]
    in_maps = []
    for cid in range(N_CORES):
        bb, mblk = cid // 4, cid % 4
        m0 = mblk * MB
        x2blk = x2[bb, m0 : m0 + MB]              # [MB, H]
        x2t = x2blk.T                             # [H, MB]
        # V3[h, m*C+c] = x2t[h,m]*W3T[h,c] + (W1-W4)T[h,c]
        v3 = x2t[:, :, None] * w3t[:, None, :] + w13[:, None, :]
        wv3 = np.concatenate([w4t2, v3.reshape(H, MB * C)], axis=1)
        in_maps.append({
            "x1t": np.ascontiguousarray(x1[bb].T.astype(BF16_NP)),
            "negx2": np.ascontiguousarray(-x2t.astype(np.float32)),
            "wv3": np.ascontiguousarray(wv3.astype(BF16_NP)),
        })
    return in_maps


def t2_bias(x2, W, b):
    """Host-side t2 term: x2 @ (W2+W4).T + bias, [B, L, C] f32."""
    W2 = W[:, H : 2 * H]
    W4 = W[:, 3 * H : 4 * H]
    return (x2 @ (W2 + W4).T + b).astype(np.float32)


def run_on_device(x1, x2, W, b, trace=False, trace_kwargs=None):
    nc = _get_compiled()
    in_maps = make_in_maps(x1, x2, W, b)
    res = bass_utils.run_bass_kernel_spmd(
        nc, in_maps, core_ids=list(range(N_CORES)), trace=trace,
        **(trace_kwargs or {}))
    t2 = t2_bias(x2, W, b)                        # [B, L, C]
    full = np.empty((B, L, L, C), dtype=np.float32)
    for cid in range(N_CORES):
        bb, mblk = cid // 4, cid % 4
        m0 = mblk * MB
        full[bb, :, m0 : m0 + MB, :] = (
            np.asarray(res.results[cid]["out"])
            .astype(np.float32).reshape(L, MB, C)
            + t2[bb, m0 : m0 + MB, :][None, :, :])
    return full, res


def kernel(x1, x2, W, b):
    x1 = np.asarray(x1, dtype=np.float32)
    x2 = np.asarray(x2, dtype=np.float32)
    W = np.asarray(W, dtype=np.float32)
    b = np.asarray(b, dtype=np.float32)
    full, _ = run_on_device(x1, x2, W, b, trace=False)
    return full


# revision 8
# speedup vs baseline: 1.0484x; 1.0066x over previous
"""Trainium2 Bass kernel for the BiaffineLayer problem.

Math (per batch b):
  out[l, m, c] = x1[l] @ W1[c] + x2[m] @ W2[c]
              + sum_h x1[l,h] * x2[m,h] * W3[c,h]
              + sum_h |x1[l,h] - x2[m,h]| * W4[c,h] + bias[c]
  shapes: x1, x2 [2, 512, 128]; W [25, 512]; bias [25]; out [2, 512, 512, 25]

Sharding: 8 cores = 2 batches x 4 m-blocks of 128 columns. Core (b, mb) gets
full x1[b] and its x2[b, m0:m0+128] block; it produces out[b, :, m0:m0+MB, :].

Decomposition, with |d| = 2*relu(d) - d and d = x1 - x2[m]:
  out = x1t' @ V3  +  D_m' @ (2 W4T)  +  T2B
where (host-precomputed except D):
  V3[h,(m,c)] = x2[m,h]*W3[c,h] + (W1-W4)[c,h]   (t3 + t1 - t4's -x1*W4 part)
  T2B[m,c]    = x2[m] @ (W2+W4)T + b             (added on the HOST during
                                                  unshard - pure per-(m,c))
  D_m[h,l]    = relu(x1[l,h] - x2[m,h])          (device, pairwise)

Measured HW rates (microbenched): DVE tensor_scalar = 129ns + 0.26ns/elem
(~263ns per [128,512] D tile, 2x bf16 mode already engaged); ACT relu ~612ns
effective, ACT [128,1600] PSUM drain ~1480ns; PE t4 LDW+MM pair 25ns, t3
400-col MM 168ns (PE total ~18us, not critical); input DMA is descriptor-
rate-bound (~17ns per 1KB partition row -> 2.2us for x1t on one queue);
gpsimd compute useless (7.5us/op); large out-DMAs on the gpsimd queue hit a
~19us descriptor pathology - keep them on sync.

Schedule (v3):
  - input DMAs split across queues to parallelize descriptor processing:
    sync: x1t[:,0:256] -> wv3a(w4t2+v3 blocks 0-1) -> out-DMAs
    vector: x1t[:,256:512] -> wv3b(v3 blocks 2-4)
    gpsimd: negx2[:,0:64] -> negx2[:,64:128] -> wv3c(v3 blocks 5-7)
    D-gen starts ~9.1us instead of ~10.6.
  - single 4-bank PSUM tile per block (bufs=2), drain = ONE ACT op
    [128,4,400] per block, emitted one block late; last block's drain
    split DVE+ACT with its two out-DMAs on sync+tensor queues.
  - D-tile engine split DVE 102 / ACT 26 (block 0-1 give ACT 4 tiles as
    they carry no/less drain work).
"""

import sys

sys.path.insert(0, "/opt/trn_rl_repo")

from contextlib import ExitStack

import ml_dtypes
import numpy as np

import concourse.bass as bass
import concourse.tile as tile
from concourse import bacc, bass_utils, mybir

F32 = mybir.dt.float32
BF16 = mybir.dt.bfloat16
BF16_NP = ml_dtypes.bfloat16

B, L, H, C = 2, 512, 128, 25
MB = 128            # m-block per core
N_CORES = 8
MSUB = 16           # m's per psum block
N_MS = MB // MSUB   # 8 blocks over the m-block
LCHUNK = 128
N_LC = L // LCHUNK  # 4 l-chunks
CHUNK_F = MSUB * C  # 400 psum free columns per l-chunk slice
PS_STRIDE = 512     # psum bank stride (f32 elems) per l-chunk slice

# Per-block D-tile engine split: V=DVE, A=ACT. sum V = 107.
# ACT's per-block queue is [drain(k-1), A-tiles...]; A positions are late
# (j=9..13) so they are ready just when the PE's j-major consumption,
# which tracks the production rate, reaches them -- but never at j=14/15,
# so the block's end never waits on the slower ACT engine.
D_PATS = {
    11: "VVAVVAVVAVVAVVAV",   # V=11 A=5 (block 0: no drain, ACT has slack)
    13: "VVVVVVVVVAVAVAVV",   # V=13 A=3 at j=9,11,13
    14: "VVVVVVVVVVVAVAVV",   # V=14 A=2 at j=11,13
}
D_NV = [11, 13, 13, 14, 14, 14, 14, 14]   # sum = 107


def build_kernel(nc: bass.Bass, repeat: int = 1):
    x1t = nc.dram_tensor("x1t", (H, L), BF16, kind="ExternalInput").ap()
    negx2 = nc.dram_tensor("negx2", (H, MB), F32, kind="ExternalInput").ap()
    # w4t2 (25 cols) | v3 (3200 cols) merged
    wv3 = nc.dram_tensor("wv3", (H, C + MB * C), BF16, kind="ExternalInput").ap()
    out = nc.dram_tensor("out", (L, MB * C), BF16, kind="ExternalOutput").ap()

    WVA = C + 2 * CHUNK_F        # w4t2 + v3 blocks 0-1
    WVB = C + 5 * CHUNK_F        # ... through block 4

    with tile.TileContext(nc) as tc, ExitStack() as ctx:
      const = ctx.enter_context(tc.tile_pool(name="const", bufs=1))
      dpool = ctx.enter_context(tc.tile_pool(name="dpool", bufs=128))
      opool = ctx.enter_context(tc.tile_pool(name="opool", bufs=8))
      psum = ctx.enter_context(tc.tile_pool(name="psum", bufs=2, space="PSUM"))
      for _rep in range(repeat):
        # ---- input loads ----
        # DMA queues are sync/scalar/gpsimd only. Input latency is
        # descriptor-count-bound: a DMA takes ~(7.4 + 9.5*KB)ns per
        # PARTITION row on its queue's ring, so the critical x1t+negx2
        # load is split by PARTITION ranges (halving rows halves ring
        # time; splitting columns would not). The wv3 DMAs are emitted
        # AFTER block 0's D ops so the per-queue batched DMA semaphores
        # gate D-gen on x1t+negx2 only. The scalar ring processes
        # descriptors while the ACT engine loads its activation table
        # (act_warm), so using that queue costs ACT nothing.
        x1t_tile = const.tile([H, L], BF16)
        negx2_tile = const.tile([H, MB], F32)
        x1t_bf = x1t_tile[:]
        negx2_f = negx2_tile[:]
        wv3_tile = const.tile([H, C + MB * C], BF16)
        w4t2_bf = wv3_tile[:, 0:C]
        v3_bf = wv3_tile[:, C:]
        ones_bf = const.tile([1, LCHUNK], BF16)

        nc.scalar.dma_start(x1t_tile[72:128, :], x1t[72:128, :])
        nc.sync.dma_start(x1t_tile[0:72, :], x1t[0:72, :])
        nc.gpsimd.dma_start(negx2_tile[0:96, :], negx2[0:96, :])
        nc.scalar.dma_start(negx2_tile[96:128, :], negx2[96:128, :])
        nc.vector.memset(ones_bf[:], 1.0)
        # preload the ACT activation table off the critical path
        act_warm = const.tile([1, LCHUNK], BF16)
        nc.scalar.activation(act_warm[:], ones_bf[:],
                             mybir.ActivationFunctionType.Relu)

        # ---- main loop over m-blocks ----
        # Drains (one ACT op each) emitted one block late so they queue
        # behind the next block's D work, prioritizing D production.
        pend = None

        def emit_drain(p):
            ms_, ps3_, last = p
            o_sb = opool.tile([LCHUNK, N_LC * CHUNK_F], BF16)
            o3 = o_sb[:].rearrange("p (lc c) -> p lc c", c=CHUNK_F)
            out3 = (out[:, ms_ * CHUNK_F : (ms_ + 1) * CHUNK_F]
                    .rearrange("(lc p) c -> p lc c", p=LCHUNK))
            if not last:
                nc.scalar.copy(o3[:], ps3_)
                nc.sync.dma_start(out3, o3)
            else:  # split engines + 2 DMAs on 2 queues for a short tail
                nc.vector.tensor_copy(o3[:, 0:2], ps3_[:, 0:2])
                nc.sync.dma_start(out3[:, 0:2], o3[:, 0:2])
                nc.scalar.copy(o3[:, 2:4], ps3_[:, 2:4])
                nc.scalar.dma_start(out3[:, 2:4], o3[:, 2:4])

        for ms in range(N_MS):
            # drain(k-1) first: it leads the ACT queue for this block so
            # the PSUM tile frees early, and ACT's A-tiles follow it
            if pend is not None:
                emit_drain(pend)
            # D tiles for this block
            dts = []
            pat = D_PATS[D_NV[ms]]
            for j in range(MSUB):
                m = ms * MSUB + j
                dt_ = dpool.tile([H, L], BF16, tag="d")
                if pat[j] == "V":
                    nc.vector.tensor_scalar(
                        dt_[:], x1t_bf, negx2_f[:, m : m + 1], 0.0,
                        op0=mybir.AluOpType.add, op1=mybir.AluOpType.max)
                else:
                    nc.scalar.activation(
                        dt_[:], x1t_bf, mybir.ActivationFunctionType.Relu,
                        bias=negx2_f[:, m : m + 1], scale=1.0)
                dts.append(dt_)

            if ms == 0:
                # wv3 staged now: the batched per-queue DMA semaphores up
                # to this point cover only x1t/negx2, so D-gen's gates
                # fire as soon as those land. w4t2+v3[0:2] first (t3 of
                # block 0 waits on it), the rest lands well before use.
                nc.sync.dma_start(wv3_tile[:, 0:WVA], wv3[:, 0:WVA])
                nc.scalar.dma_start(wv3_tile[:, WVA:WVB], wv3[:, WVA:WVB])
                nc.gpsimd.dma_start(wv3_tile[:, WVB:], wv3[:, WVB:])

            ps = psum.tile([LCHUNK, N_LC * PS_STRIDE], F32, tag="ps")
            # t3 (+t1 fold) opens each group full-width (PSUM zeroing is
            # bank-granular), then the t4s accumulate j-major.
            for lc in range(N_LC):
                nc.tensor.matmul(
                    ps[:, lc * PS_STRIDE : lc * PS_STRIDE + CHUNK_F],
                    x1t_bf[:, lc * LCHUNK : (lc + 1) * LCHUNK],
                    v3_bf[:, ms * CHUNK_F : (ms + 1) * CHUNK_F],
                    start=True, stop=False, skip_group_check=True)
            for j in range(MSUB):
                for lc in range(N_LC):
                    base = lc * PS_STRIDE
                    nc.tensor.matmul(
                        ps[:, base + j * C : base + (j + 1) * C],
                        dts[j][:, lc * LCHUNK : (lc + 1) * LCHUNK],
                        w4t2_bf[:],
                        start=False, stop=(j == MSUB - 1),
                        skip_group_check=True)

            ps3 = ps[:].rearrange("p (lc x) -> p lc x",
                                  x=PS_STRIDE)[:, :, 0:CHUNK_F]
            pend = (ms, ps3, ms == N_MS - 1)
        emit_drain(pend)
    return nc


_COMPILED = {}


def _get_compiled():
    if "nc" not in _COMPILED:
        nc = bacc.Bacc("TRN2", target_bir_lowering=False, debug=False,
                       num_devices=N_CORES)
        build_kernel(nc)
        nc.compile()
        _COMPILED["nc"] = nc
    return _COMPILED["nc"]


def make_in_maps(x1, x2, W, b):
    W1, W2, W3, W4 = (W[:, 0:H], W[:, H : 2 * H], W[:, 2 * H : 3 * H],
                      W[:, 3 * H : 4 * H])
    w13 = (W1 - W4).T.astype(np.float32)          # [H, C]
    w3t = W3.T.astype(np.float32)                 # [H, C]
    w4t2 = (2.0 * W4).T.astype(np.float32)        # [H, C]
    in_maps = []
    for cid in range(N_CORES):
        bb, mblk = cid // 4, cid % 4
        m0 = mblk * MB
        x2blk = x2[bb, m0 : m0 + MB]              # [MB, H]
        x2t = x2blk.T                             # [H, MB]
        # V3[h, m*C+c] = x2t[h,m]*W3T[h,c] + (W1-W4)T[h,c]
        v3 = x2t[:, :, None] * w3t[:, None, :] + w13[:, None, :]
        wv3 = np.concatenate([w4t2, v3.reshape(H, MB * C)], axis=1)
        in_maps.append({
            "x1t": np.ascontiguousarray(x1[bb].T.astype(BF16_NP)),
            "negx2": np.ascontiguousarray(-x2t.astype(np.float32)),
            "wv3": np.ascontiguousarray(wv3.astype(BF16_NP)),
        })
    return in_maps


def t2_bias(x2, W, b):
    """Host-side t2 term: x2 @ (W2+W4).T + bias, [B, L, C] f32."""
    W2 = W[:, H : 2 * H]
    W4 = W[:, 3 * H : 4 * H]
    return (x2 @ (W2 + W4).T + b).astype(np.float32)


def run_on_device(x1, x2, W, b, trace=False, trace_kwargs=None):
    nc = _get_compiled()
    in_maps = make_in_maps(x1, x2, W, b)
    res = bass_utils.run_bass_kernel_spmd(
        nc, in_maps, core_ids=list(range(N_CORES)), trace=trace,
        **(trace_kwargs or {}))
    t2 = t2_bias(x2, W, b)                        # [B, L, C]
    full = np.empty((B, L, L, C), dtype=np.float32)
    for cid in range(N_CORES):
        bb, mblk = cid // 4, cid % 4
        m0 = mblk * MB
        full[bb, :, m0 : m0 + MB, :] = (
            np.asarray(res.results[cid]["out"])
            .astype(np.float32).reshape(L, MB, C)
            + t2[bb, m0 : m0 + MB, :][None, :, :])
    return full, res


def kernel(x1, x2, W, b):
    x1 = np.asarray(x1, dtype=np.float32)
    x2 = np.asarray(x2, dtype=np.float32)
    W = np.asarray(W, dtype=np.float32)
    b = np.asarray(b, dtype=np.float32)
    full, _ = run_on_device(x1, x2, W, b, trace=False)
    return full


# revision 12
# speedup vs baseline: 1.1471x; 1.0942x over previous
"""Trainium2 Bass kernel for the BiaffineLayer problem.

Math (per batch b):
  out[l, m, c] = x1[l] @ W1[c] + x2[m] @ W2[c]
              + sum_h x1[l,h] * x2[m,h] * W3[c,h]
              + sum_h |x1[l,h] - x2[m,h]| * W4[c,h] + bias[c]
  shapes: x1, x2 [2, 512, 128]; W [25, 512]; bias [25]; out [2, 512, 512, 25]

Sharding: 8 cores = 2 batches x 4 m-blocks of 128 columns. Core (b, mb) gets
full x1[b] and its x2[b, m0:m0+128] block; it produces out[b, :, m0:m0+MB, :].

Decomposition, with |d| = 2*relu(d) - d and d = x1 - x2[m]:
  out = x1t' @ V3  +  D_m' @ (2 W4T)  +  T2B
where (host-precomputed except D):
  V3[h,(m,c)] = x2[m,h]*W3[c,h] + (W1-W4)[c,h]   (t3 + t1 - t4's -x1*W4 part)
  T2B[m,c]    = x2[m] @ (W2+W4)T + b             (added on the HOST during
                                                  unshard - pure per-(m,c))
  D_m[h,l]    = relu(x1[l,h] - x2[m,h])          (device, pairwise)

Measured HW rates (microbenched): DVE tensor_scalar = 129ns + 0.26ns/elem
(~263ns per [128,512] D tile, 2x bf16 mode already engaged); ACT relu ~612ns
effective, ACT [128,1600] PSUM drain ~1480ns; PE t4 LDW+MM pair 25ns, t3
400-col MM 168ns (PE total ~18us, not critical); input DMA is descriptor-
rate-bound (~17ns per 1KB partition row -> 2.2us for x1t on one queue);
gpsimd compute useless (7.5us/op); large out-DMAs on the gpsimd queue hit a
~19us descriptor pathology - keep them on sync.

Schedule (v3):
  - input DMAs split across queues to parallelize descriptor processing:
    sync: x1t[:,0:256] -> wv3a(w4t2+v3 blocks 0-1) -> out-DMAs
    vector: x1t[:,256:512] -> wv3b(v3 blocks 2-4)
    gpsimd: negx2[:,0:64] -> negx2[:,64:128] -> wv3c(v3 blocks 5-7)
    D-gen starts ~9.1us instead of ~10.6.
  - single 4-bank PSUM tile per block (bufs=2), drain = ONE ACT op
    [128,4,400] per block, emitted one block late; last block's drain
    split DVE+ACT with its two out-DMAs on sync+tensor queues.
  - D-tile engine split DVE 102 / ACT 26 (block 0-1 give ACT 4 tiles as
    they carry no/less drain work).
"""

import sys

sys.path.insert(0, "/opt/trn_rl_repo")

from contextlib import ExitStack

import ml_dtypes
import numpy as np

import concourse.bass as bass
import concourse.tile as tile
from concourse import bacc, bass_utils, mybir

F32 = mybir.dt.float32
BF16 = mybir.dt.bfloat16
BF16_NP = ml_dtypes.bfloat16

B, L, H, C = 2, 512, 128, 25
MB = 128            # m-block per core
N_CORES = 8
MSUB = 16           # m's per psum block
N_MS = MB // MSUB   # 8 blocks over the m-block
LCHUNK = 128
N_LC = L // LCHUNK  # 4 l-chunks
CHUNK_F = MSUB * C  # 400 psum free columns per l-chunk slice
PS_STRIDE = 512     # psum bank stride (f32 elems) per l-chunk slice

# Per-block D-tile engine split: V=DVE, A=ACT. sum V = 108.
# ACT's per-block queue is [A-tiles..., drain(k-1)]; A positions are EARLY
# (j=1,3,..) so the block's end never waits on the slower ACT engine --
# the cadence is then set purely by DVE (nV x 263ns). The drain runs in
# ACT's idle tail of each block. First and last blocks are ACT-heavy:
# block 0 has no drain yet, and a DVE-light final block shortens the tail
# (DVE finishes early and takes half of the final drain).
D_PATS = {
    12: "VAVAVAVAVVVVVVVV",   # V=12 A=4 at j=1,3,5,7
    14: "VAVAVVVVVVVVVVVV",   # V=14 A=2 at j=1,3
}
D_NV = [12, 14, 14, 14, 14, 14, 14, 12]   # sum = 108


def build_kernel(nc: bass.Bass, repeat: int = 1):
    x1t = nc.dram_tensor("x1t", (H, L), BF16, kind="ExternalInput").ap()
    negx2 = nc.dram_tensor("negx2", (H, MB), F32, kind="ExternalInput").ap()
    # w4t2 (25 cols) | v3 (3200 cols) merged
    wv3 = nc.dram_tensor("wv3", (H, C + MB * C), BF16, kind="ExternalInput").ap()
    out = nc.dram_tensor("out", (L, MB * C), BF16, kind="ExternalOutput").ap()

    WVA = C + 2 * CHUNK_F        # w4t2 + v3 blocks 0-1
    WVB = C + 5 * CHUNK_F        # ... through block 4

    with tile.TileContext(nc) as tc, ExitStack() as ctx:
      const = ctx.enter_context(tc.tile_pool(name="const", bufs=1))
      dpool = ctx.enter_context(tc.tile_pool(name="dpool", bufs=128))
      opool = ctx.enter_context(tc.tile_pool(name="opool", bufs=8))
      psum = ctx.enter_context(tc.tile_pool(name="psum", bufs=2, space="PSUM"))
      for _rep in range(repeat):
        # ---- input loads ----
        # DMA queues are sync/scalar/gpsimd only. Input latency is
        # descriptor-count-bound: a DMA takes ~(7.4 + 9.5*KB)ns per
        # PARTITION row on its queue's ring, so the critical x1t+negx2
        # load is split by PARTITION ranges (halving rows halves ring
        # time; splitting columns would not). The wv3 DMAs are emitted
        # AFTER block 0's D ops so the per-queue batched DMA semaphores
        # gate D-gen on x1t+negx2 only. The scalar ring processes
        # descriptors while the ACT engine loads its activation table
        # (act_warm), so using that queue costs ACT nothing.
        x1t_tile = const.tile([H, L], BF16)
        negx2_tile = const.tile([H, MB], F32)
        x1t_bf = x1t_tile[:]
        negx2_f = negx2_tile[:]
        wv3_tile = const.tile([H, C + MB * C], BF16)
        w4t2_bf = wv3_tile[:, 0:C]
        v3_bf = wv3_tile[:, C:]
        ones_bf = const.tile([1, LCHUNK], BF16)

        # Every tensor D-gen needs is FIRST on its ring: a ring's second
        # DMA starts ~1-2us after the first finishes, so nothing block 0
        # depends on may ride second.
        nc.scalar.dma_start(x1t_tile[64:128, :], x1t[64:128, :])
        nc.sync.dma_start(x1t_tile[0:64, :], x1t[0:64, :])
        nc.gpsimd.dma_start(negx2_tile[:], negx2[:])
        nc.vector.memset(ones_bf[:], 1.0)
        # preload the ACT activation table off the critical path
        act_warm = const.tile([1, LCHUNK], BF16)
        nc.scalar.activation(act_warm[:], ones_bf[:],
                             mybir.ActivationFunctionType.Relu)

        # ---- main loop over m-blocks ----
        # Drains (one ACT op each) emitted one block late so they queue
        # behind the next block's D work, prioritizing D production.
        pend = None

        def emit_drain(p):
            ms_, ps3_, last = p
            o_sb = opool.tile([LCHUNK, N_LC * CHUNK_F], BF16)
            o3 = o_sb[:].rearrange("p (lc c) -> p lc c", c=CHUNK_F)
            out3 = (out[:, ms_ * CHUNK_F : (ms_ + 1) * CHUNK_F]
                    .rearrange("(lc p) c -> p lc c", p=LCHUNK))
            if not last:
                nc.scalar.copy(o3[:], ps3_)
                nc.sync.dma_start(out3, o3)
            else:  # split engines + 2 DMAs on 2 queues for a short tail
                nc.vector.tensor_copy(o3[:, 0:2], ps3_[:, 0:2])
                nc.sync.dma_start(out3[:, 0:2], o3[:, 0:2])
                nc.scalar.copy(o3[:, 2:4], ps3_[:, 2:4])
                nc.scalar.dma_start(out3[:, 2:4], o3[:, 2:4])

        for ms in range(N_MS):
            # D tiles for this block
            dts = []
            pat = D_PATS[D_NV[ms]]
            for j in range(MSUB):
                m = ms * MSUB + j
                dt_ = dpool.tile([H, L], BF16, tag="d")
                if pat[j] == "V":
                    nc.vector.tensor_scalar(
                        dt_[:], x1t_bf, negx2_f[:, m : m + 1], 0.0,
                        op0=mybir.AluOpType.add, op1=mybir.AluOpType.max)
                else:
                    nc.scalar.activation(
                        dt_[:], x1t_bf, mybir.ActivationFunctionType.Relu,
                        bias=negx2_f[:, m : m + 1], scale=1.0)
                dts.append(dt_)

            if ms == 0:
                # wv3 rides second on each ring (behind the x1t/negx2
                # pieces); the per-queue FIFO rings mean D-gen's gates
                # still fire right when x1t/negx2 land. w4t2+v3[0:2]
                # first (block 0's t3 waits on it, ~13.5us).
                nc.sync.dma_start(wv3_tile[:, 0:WVA], wv3[:, 0:WVA])
                nc.scalar.dma_start(wv3_tile[:, WVA:WVB], wv3[:, WVA:WVB])
                nc.gpsimd.dma_start(wv3_tile[:, WVB:], wv3[:, WVB:])
            if pend is not None:
                emit_drain(pend)

            ps = psum.tile([LCHUNK, N_LC * PS_STRIDE], F32, tag="ps")
            # t3 (+t1 fold) opens each group full-width (PSUM zeroing is
            # bank-granular), then the t4s accumulate j-major.
            for lc in range(N_LC):
                nc.tensor.matmul(
                    ps[:, lc * PS_STRIDE : lc * PS_STRIDE + CHUNK_F],
                    x1t_bf[:, lc * LCHUNK : (lc + 1) * LCHUNK],
                    v3_bf[:, ms * CHUNK_F : (ms + 1) * CHUNK_F],
                    start=True, stop=False, skip_group_check=True)
            for j in range(MSUB):
                for lc in range(N_LC):
                    base = lc * PS_STRIDE
                    nc.tensor.matmul(
                        ps[:, base + j * C : base + (j + 1) * C],
                        dts[j][:, lc * LCHUNK : (lc + 1) * LCHUNK],
                        w4t2_bf[:],
                        start=False, stop=(j == MSUB - 1),
                        skip_group_check=True)

            ps3 = ps[:].rearrange("p (lc x) -> p lc x",
                                  x=PS_STRIDE)[:, :, 0:CHUNK_F]
            pend = (ms, ps3, ms == N_MS - 1)
        emit_drain(pend)
    return nc


_COMPILED = {}


def _get_compiled():
    if "nc" not in _COMPILED:
        nc = bacc.Bacc("TRN2", target_bir_lowering=False, debug=False,
                       num_devices=N_CORES)
        build_kernel(nc)
        nc.compile()
        _COMPILED["nc"] = nc
    return _COMPILED["nc"]


def make_in_maps(x1, x2, W, b):
    W1, W2, W3, W4 = (W[:, 0:H], W[:, H : 2 * H], W[:, 2 * H : 3 * H],
                      W[:, 3 * H : 4 * H])
    w13 = (W1 - W4).T.astype(np.float32)          # [H, C]
    w3t = W3.T.astype(np.float32)                 # [H, C]
    w4t2 = (2.0 * W4).T.astype(np.float32)        # [H, C]
    in_maps = []
    for cid in range(N_CORES):
        bb, mblk = cid // 4, cid % 4
        m0 = mblk * MB
        x2blk = x2[bb, m0 : m0 + MB]              # [MB, H]
        x2t = x2blk.T                             # [H, MB]
        # V3[h, m*C+c] = x2t[h,m]*W3T[h,c] + (W1-W4)T[h,c]
        v3 = x2t[:, :, None] * w3t[:, None, :] + w13[:, None, :]
        wv3 = np.concatenate([w4t2, v3.reshape(H, MB * C)], axis=1)
        in_maps.append({
            "x1t": np.ascontiguousarray(x1[bb].T.astype(BF16_NP)),
            "negx2": np.ascontiguousarray(-x2t.astype(np.float32)),
            "wv3": np.ascontiguousarray(wv3.astype(BF16_NP)),
        })
    return in_maps


def t2_bias(x2, W, b):
    """Host-side t2 term: x2 @ (W2+W4).T + bias, [B, L, C] f32."""
    W2 = W[:, H : 2 * H]
    W4 = W[:, 3 * H : 4 * H]
    return (x2 @ (W2 + W4).T + b).astype(np.float32)


def run_on_device(x1, x2, W, b, trace=False, trace_kwargs=None):
    nc = _get_compiled()
    in_maps = make_in_maps(x1, x2, W, b)
    res = bass_utils.run_bass_kernel_spmd(
        nc, in_maps, core_ids=list(range(N_CORES)), trace=trace,
        **(trace_kwargs or {}))
    t2 = t2_bias(x2, W, b)                        # [B, L, C]
    full = np.empty((B, L, L, C), dtype=np.float32)
    for cid in range(N_CORES):
        bb, mblk = cid // 4, cid % 4
        m0 = mblk * MB
        full[bb, :, m0 : m0 + MB, :] = (
            np.asarray(res.results[cid]["out"])
            .astype(np.float32).reshape(L, MB, C)
            + t2[bb, m0 : m0 + MB, :][None, :, :])
    return full, res


def kernel(x1, x2, W, b):
    x1 = np.asarray(x1, dtype=np.float32)
    x2 = np.asarray(x2, dtype=np.float32)
    W = np.asarray(W, dtype=np.float32)
    b = np.asarray(b, dtype=np.float32)
    full, _ = run_on_device(x1, x2, W, b, trace=False)
    return full
